# revision 1
# baseline (speedup 1.0000x reference)
"""Trainium2 Bass kernel for nn_PriorW (Wishart-prior sampling).

Math per batch b (wdim=16):
  A  = tril(A_noise,-1) + diag(sqrt(chisq))           (lower-triangular)
  B  = inv(A)                                          (lower-triangular)
  scale_i = 1/sqrt(sum_j B[j,i]^2)                     (col norms of B)
  M^T[j,i] = B[j,i]*scale_i
  w[b,n,:] = z[b,n,:] @ M^T                            (n = 4096 draws)

Sharding: pure data parallel over nbatch (1024 / 8 cores = 128 per core).
On each core the 128 batches are packed 8-at-a-time into block-diagonal
128x128 stationary matrices for the PE (contraction dim = (slot, j)), and
z is fed pre-transposed so its j index lands on partitions:
  group g in [0,16), slot b8 in [0,8): batch b = 16*b8 + g
  zt row (g*128 + 16*b8 + j) = z[b, :, j]   (n contiguous -> fast DMA)
  wt row (g*128 + 16*b8 + i) = w[b, :, i]
"""
import os
import sys
import types

import numpy as np

for _p in ("/opt/trn_rl_repo", "/root/.axon_site/_ro/trn_rl_repo"):
    if _p not in sys.path:
        sys.path.append(_p)

import concourse.bass as bass  # noqa: E402
import concourse.tile as tile  # noqa: E402
from concourse import bacc, mybir  # noqa: E402
from concourse import bass_utils  # noqa: E402

WD = 16          # wishart dim
BPC = 128        # batches per core
N = 4096         # draws per batch
NT = 512         # matmul moving-dim tile (fp32 max)
NCORES = 8
NBATCH = BPC * NCORES
F32 = mybir.dt.float32

_PROGRAM_CACHE = {}


def _setup_trace_hooks():
    """Register the axon NTFF profile hook (missing antenv.axon_hooks shim)."""
    try:
        import antenv
        if "antenv.axon_hooks" not in sys.modules:
            hooks = types.ModuleType("antenv.axon_hooks")
            _h = [None]
            hooks.set_axon_ntff_profile_hook = lambda h: _h.__setitem__(0, h)
            hooks.get_axon_ntff_profile_hook = lambda: _h[0]
            sys.modules["antenv.axon_hooks"] = hooks
            antenv.axon_hooks = hooks
        from antenv.axon_hooks import set_axon_ntff_profile_hook
        from trn_agent_boot.trn_boot import _ntff_profile_via_ctypes
        hook = _ntff_profile_via_ctypes("/opt/axon/libaxon_pjrt.so")
        if hook is not None:
            set_axon_ntff_profile_hook(hook)
        bass_utils.upload_artifacts = lambda tmpdir: tmpdir  # no egress
        return True
    except Exception:
        return False


def _build_program():
    nc = bacc.Bacc("TRN2", target_bir_lowering=False)
    an_ext = nc.declare_dram_parameter("a_noise", [BPC, WD * WD], F32, isOutput=False)
    cs_ext = nc.declare_dram_parameter("chisq", [BPC, WD], F32, isOutput=False)
    zt_ext = nc.declare_dram_parameter("zt", [BPC * WD, N], F32, isOutput=False)
    wt_ext = nc.declare_dram_parameter("wt", [BPC * WD, N], F32, isOutput=True)
    # bounce buffer for the partition<->free shuffle of M^T: [b8, j, g, i]
    mt_dram = nc.dram_tensor("mt_scratch", [8, WD, WD, WD], F32)

    MULT = mybir.AluOpType.mult
    ADD = mybir.AluOpType.add

    with tile.TileContext(nc) as tc:
        with tc.tile_pool(name="pro", bufs=1) as pro, \
             tc.tile_pool(name="zin", bufs=3) as zpool, \
             tc.tile_pool(name="wout", bufs=3) as wpool, \
             tc.tile_pool(name="ps", bufs=6, space="PSUM") as pspool:

            # ---------- prologue: per-batch 16x16 factor prep ----------
            an = pro.tile([BPC, WD * WD], F32)
            nc.sync.dma_start(out=an, in_=an_ext[:])
            cs = pro.tile([BPC, WD], F32)
            nc.sync.dma_start(out=cs, in_=cs_ext[:])

            d = pro.tile([BPC, WD], F32)
            nc.scalar.sqrt(d, cs)
            r = pro.tile([BPC, WD], F32)
            nc.vector.reciprocal(r, d)
            nr = pro.tile([BPC, WD], F32)
            nc.vector.tensor_scalar_mul(nr, r, -1.0)

            # forward substitution: row i of B = (e_i - sum_{k<i} A[i,k]*B_k)/d_i
            Bm = pro.tile([BPC, WD * WD], F32)
            acc = pro.tile([BPC, WD], F32)
            nc.vector.memset(Bm[:, 0:WD], 0.0)
            nc.vector.tensor_copy(Bm[:, 0:1], r[:, 0:1])
            for i in range(1, WD):
                nc.vector.tensor_scalar_mul(
                    acc, Bm[:, 0:WD], an[:, WD * i:WD * i + 1])
                for k in range(1, i):
                    nc.vector.scalar_tensor_tensor(
                        out=acc,
                        in0=Bm[:, WD * k:WD * (k + 1)],
                        scalar=an[:, WD * i + k:WD * i + k + 1],
                        in1=acc,
                        op0=MULT,
                        op1=ADD,
                    )
                nc.vector.tensor_scalar_mul(
                    Bm[:, WD * i:WD * (i + 1)], acc, nr[:, i:i + 1])
                nc.vector.tensor_copy(
                    Bm[:, WD * i + i:WD * i + i + 1], r[:, i:i + 1])

            # column norms -> scale -> M^T = B * scale (broadcast over rows)
            sq = pro.tile([BPC, WD * WD], F32)
            nc.vector.tensor_mul(sq, Bm, Bm)
            s2 = pro.tile([BPC, WD], F32)
            nc.vector.tensor_copy(s2, sq[:, 0:WD])
            for j in range(1, WD):
                nc.vector.tensor_add(s2, s2, sq[:, WD * j:WD * (j + 1)])
            ssq = pro.tile([BPC, WD], F32)
            nc.scalar.sqrt(ssq, s2)
            scl = pro.tile([BPC, WD], F32)
            nc.vector.reciprocal(scl, ssq)
            mt = pro.tile([BPC, WD * WD], F32)
            for j in range(WD):
                nc.vector.tensor_mul(
                    mt[:, WD * j:WD * (j + 1)], Bm[:, WD * j:WD * (j + 1)], scl)

            # ---------- build 16 block-diagonal stationaries ----------
            # lhsT_all[:, 128g:128(g+1)][16*b8+j, 16*b8+i] = mt[16*b8+g][j,i]
            lhsT_all = pro.tile([BPC, 16 * BPC], F32)
            nc.vector.memset(lhsT_all, 0.0)
            for b8 in range(8):
                psl = slice(16 * b8, 16 * (b8 + 1))
                # SBUF[g-part, (j i)] -> DRAM[b8][j, g, i]
                nc.sync.dma_start(
                    out=mt_dram[b8].rearrange("j g i -> g j i"),
                    in_=mt[psl, :].rearrange("g (j i) -> g j i", j=WD),
                )
            for b8 in range(8):
                psl = slice(16 * b8, 16 * (b8 + 1))
                dst = lhsT_all[psl, :].rearrange("j (g f) -> j g f", g=16)
                nc.sync.dma_start(
                    out=dst[:, :, 16 * b8:16 * (b8 + 1)],
                    in_=mt_dram[b8],
                )

            # ---------- main stream: w^T = blockdiag(M)^T @ z^T ----------
            for g in range(16):
                rsl = slice(g * BPC, (g + 1) * BPC)
                zin = zpool.tile([BPC, N], F32)
                nc.sync.dma_start(out=zin, in_=zt_ext[rsl, :])
                wout = wpool.tile([BPC, N], F32)
                for t in range(N // NT):
                    csl = slice(NT * t, NT * (t + 1))
                    ps = pspool.tile([BPC, NT], F32)
                    nc.tensor.matmul(
                        ps,
                        lhsT=lhsT_all[:, BPC * g:BPC * (g + 1)],
                        rhs=zin[:, csl],
                        start=True,
                        stop=True,
                    )
                    nc.vector.tensor_copy(wout[:, csl], ps)
                nc.scalar.dma_start(out=wt_ext[rsl, :], in_=wout)

    nc.compile()
    return nc


def _get_program():
    if "nc" not in _PROGRAM_CACHE:
        _PROGRAM_CACHE["nc"] = _build_program()
    return _PROGRAM_CACHE["nc"]


def kernel(A_noise, chisq, z):
    from concourse.bass_utils import run_bass_kernel_spmd

    A_noise = np.ascontiguousarray(A_noise, dtype=np.float32)
    chisq = np.ascontiguousarray(chisq, dtype=np.float32)
    z = np.ascontiguousarray(z, dtype=np.float32)

    trace = bool(os.environ.get("PRIORW_TRACE"))
    if trace:
        trace = _setup_trace_hooks()

    nc = _get_program()

    in_maps = []
    for c in range(NCORES):
        sl = slice(c * BPC, (c + 1) * BPC)
        # [128,4096,16] -> (b8, g, n, j) -> (g, b8, j, n) -> [2048, 4096]
        zt = np.ascontiguousarray(
            z[sl].reshape(8, 16, N, WD).transpose(1, 0, 3, 2)
        ).reshape(BPC * WD, N)
        in_maps.append({
            "a_noise": A_noise[sl].reshape(BPC, WD * WD),
            "chisq": chisq[sl],
            "zt": zt,
        })

    res = run_bass_kernel_spmd(nc, in_maps, list(range(NCORES)), trace=trace)
    if trace and res.exec_time_ns is not None:
        print(f"HW exec time: {res.exec_time_ns} ns")

    w = np.empty((NBATCH, N, WD), dtype=np.float32)
    for c in range(NCORES):
        wt = np.asarray(res.results[c]["wt"])
        w[c * BPC:(c + 1) * BPC] = (
            wt.reshape(16, 8, WD, N).transpose(1, 0, 3, 2).reshape(BPC, N, WD)
        )
    return w
